# revision 32
# baseline (speedup 1.0000x reference)
"""Trainium2 Bass kernel for CustomMultiHeadAttention (B=4, S=1024, D=1024, H=16, Dh=64).

Sharding: 8 cores = (batch b in 0..3) x (head-half hh in 0..1).
Core (b, hh) computes heads 8*hh..8*hh+7 of batch b over the FULL
sequence (natural q order), producing a partial output
y_part = ctx_half @ Wo[512*hh:512*hh+512, :]; the host sums the two
partials per batch (and adds bo).  This halves projection FLOPs and
weight DMA versus data-parallel-over-queries.

Pipeline (transposed layout, PE-centric):
  QT = rope(Wq^T x^T), KT = rope(Wk^T x^T)  - rope via permutation-matmul + DVE
  per head pair p (4) x q-chunk n (2): scT[kv,q] = KT_h^T QT_h,
  exp on ScalarE (scale=1/8), causal mask on the diagonal 128-col block,
  ctx accumulates with lhsT = [V_h0|1|V_h1] slots: h0 gets a free
  denominator row (M=65), h1's denominator is one M=1 matmul into the
  unused row 0 of its ctx PSUM bank.  Normalization reciprocals are
  partition-broadcast on GpSimd (no PE broadcast matmul), then
  cn = ctx * recip on DVE; out = cn^T Wo_half.
"""

import threading

import numpy as np

B, S, D, H, Dh = 4, 1024, 1024, 16, 64
P = 128
N_CORES = 8
KT = 8    # k (din) tiles
HT = 4    # dout tiles per core (8 heads = 512 dims)
ST = 8    # s tiles
VS2 = 130  # V pair slot: [V_h0(64) | 1 | V_h1(64) | 1]

_cache = {}
_lock = threading.Lock()


def _build_program(taps=False):
    import concourse.bass as bass  # noqa: F401
    import concourse.mybir as mybir
    import concourse.tile as tile
    from concourse import bacc

    dt = mybir.dt
    f16, f32 = dt.float16, dt.float32
    AF = mybir.ActivationFunctionType

    nc = bacc.Bacc("TRN2", target_bir_lowering=False, debug=False,
                   num_devices=N_CORES)

    def ein(name, shape):
        return nc.dram_tensor(name, shape, f16, kind="ExternalInput").ap()

    xt_sh = ein("xt_sh", [P, KT, S])      # x[b]^T, host-transposed
    wq_e = ein("wq", [D, 512])            # Wq[:, half-cols]
    wk_e = ein("wk", [D, 512])
    wv_e = ein("wv", [D, 512])
    wo_e = ein("wo", [512, D])            # Wo[half-rows, :]
    bqt_e = nc.dram_tensor("bqt", [P, HT], f32, kind="ExternalInput").ap()
    bkt_e = nc.dram_tensor("bkt", [P, HT], f32, kind="ExternalInput").ap()
    bv_e = ein("bv", [1, 512])
    cos_e = ein("cosT", [P, S])
    sin_e = ein("sinT", [P, S])
    mj_e = ein("mj", [P, P])
    p128_e = ein("p128", [P, P])
    y_sh = nc.dram_tensor("y_sh", [S, D], f16, kind="ExternalOutput").ap()
    tap_ext = {}
    if taps:
        for tn, shape in (("qt", [P, HT, S]), ("kt", [P, HT, S]),
                          ("v1", [P, ST, HT * VS2]), ("cn", [P, HT, S])):
            tap_ext[tn] = nc.dram_tensor("dbg_" + tn, shape, f16,
                                         kind="ExternalOutput").ap()
        tap_ext["nrm"] = nc.dram_tensor("dbg_nrm", [64, 8, 1024], f32,
                                        kind="ExternalOutput").ap()
        tap_ext["den"] = nc.dram_tensor("dbg_den", [8, 1024], f32,
                                        kind="ExternalOutput").ap()
        tap_ext["r01"] = nc.dram_tensor("dbg_r01", [8, 1024], f32,
                                        kind="ExternalOutput").ap()

    with tile.TileContext(nc) as tc:
        from contextlib import ExitStack
        with ExitStack() as ctx:
            big = ctx.enter_context(tc.tile_pool(name="big", bufs=1))

            xT = big.tile([P, KT, S], f16, tag="xT")       # x^T  [din, s]
            wq = big.tile([P, KT, 512], f16, tag="wq")
            wk = big.tile([P, KT, 512], f16, tag="wk")
            wv = big.tile([P, KT, 512], f16, tag="wv")
            wo = big.tile([P, HT, D], f16, tag="wo")
            bqt = big.tile([P, HT], f32, tag="bqt")
            bkt = big.tile([P, HT], f32, tag="bkt")
            bv_sb = big.tile([1, 512], f16, tag="bv")
            qt = big.tile([P, HT, S], f16, tag="qt")       # rope'd Q^T
            kt = big.tile([P, HT, S], f16, tag="kt")       # rope'd K^T
            v1 = big.tile([P, ST, HT * VS2], f16, tag="v1")
            cn = big.tile([P, HT, S], f16, tag="cn")       # normalized ctx^T
            cosT = big.tile([P, S], f16, tag="cosT")
            sinT = big.tile([P, S], f16, tag="sinT")
            mj = big.tile([P, P], f16, tag="mj")
            p128 = big.tile([P, P], f16, tag="p128")
            ones = big.tile([P, P], f16, tag="ones")

            # ---- input DMAs ----
            # sync queue: strict priority order - wq[k] and the c0 column
            # half of x[k] pairwise so the first Q rope chains start ~3us
            # in; everything small rides the gpsimd queue in parallel.
            for k in range(KT):
                nc.sync.dma_start(wq[:, k, :], wq_e[P * k:P * (k + 1), :])
                nc.sync.dma_start(xT[:, k, 0:512], xt_sh[:, k, 0:512])
            for k in range(KT):
                nc.sync.dma_start(xT[:, k, 512:1024], xt_sh[:, k, 512:1024])
            for k in range(KT):
                nc.sync.dma_start(wk[:, k, :], wk_e[P * k:P * (k + 1), :])
            for k in range(KT):
                nc.sync.dma_start(wv[:, k, :], wv_e[P * k:P * (k + 1), :])
            nc.sync.dma_start(bv_sb[:], bv_e[:])
            for t in range(HT):
                nc.sync.dma_start(wo[:, t, :], wo_e[P * t:P * (t + 1), :])
            for t, e in ((p128, p128_e), (cosT, cos_e), (sinT, sin_e),
                         (bqt, bqt_e), (bkt, bkt_e), (mj, mj_e)):
                nc.gpsimd.dma_start(t[:], e[:])
            nc.any.memset(ones[:], 1.0)
            v1r = v1.rearrange("p t (pr c) -> p t pr c", c=VS2)
            nc.any.memset(v1r[:, :, :, 64:65], 1.0)
            nc.any.memset(v1r[:, :, :, 129:130], 1.0)

            # ---- projections + rope, software-pipelined per column-half --
            with tc.tile_pool(name="pp", bufs=3, space="PSUM") as pp, \
                 tc.tile_pool(name="sc", bufs=4) as sc:
                # stage 1: k-chain matmuls into ps; stage 2 (lagged by one
                # block): perm matmul + rope DVE tail, so the PE never waits
                # on the DVE cast of the current block.
                blocks = []
                for c in range(2):
                    for t in range(HT):
                        blocks.append(("q", t, c))
                for c in range(2):
                    for t in range(HT):
                        blocks.append(("k", t, c))
                for t in range(ST):
                    blocks.append(("v", t, 0))

                state = {}

                def proj_stage1(blk):
                    kind, t, c = blk
                    csl = slice(512 * c, 512 * (c + 1))
                    dst_sl = slice(P * t, P * (t + 1))
                    ps = pp.tile([P, 512], f32, tag="ps", name="ps")
                    if kind == "v":
                        for k in range(KT):
                            nc.tensor.matmul(ps[:], xT[:, k, dst_sl],
                                             wv[:, k, :],
                                             start=(k == 0), stop=False)
                        nc.tensor.matmul(ps[:], ones[0:1, 0:P],
                                         bv_sb[0:1, :],
                                         start=False, stop=True)
                        state[blk] = ps
                        return
                    w_sb = wq if kind == "q" else wk
                    bias = bqt if kind == "q" else bkt
                    for k in range(KT):
                        nc.tensor.matmul(ps[:], w_sb[:, k, dst_sl],
                                         xT[:, k, csl],
                                         start=(k == 0), stop=(k == KT - 1))
                    raw = sc.tile([P, 512], f16, tag="raw", name="raw")
                    nc.vector.tensor_scalar_add(raw[:], ps[:],
                                                bias[:, t:t + 1])
                    state[blk] = raw

                def proj_stage2(blk):
                    kind, t, c = blk
                    csl = slice(512 * c, 512 * (c + 1))
                    if kind == "v":
                        vp = state.pop(blk)
                        vpr = vp.rearrange("p (pr two cc) -> p pr two cc",
                                           two=2, cc=64)
                        nc.vector.tensor_copy(v1r[:, t, :, 0:64],
                                              vpr[:, :, 0, :])
                        nc.vector.tensor_copy(v1r[:, t, :, 65:129],
                                              vpr[:, :, 1, :])
                        return
                    raw = state.pop(blk)
                    dst = (qt if kind == "q" else kt)[:, t, csl]
                    pq = pp.tile([P, 512], f32, tag="pq", name="pq")
                    nc.tensor.matmul(pq[:], p128[:], raw[:],
                                     start=True, stop=True)
                    t1 = sc.tile([P, 512], f16, tag="t1", name="t1")
                    nc.vector.tensor_mul(t1[:], raw[:], cosT[:, csl])
                    t2 = sc.tile([P, 512], f16, tag="t2", name="t2")
                    nc.vector.tensor_mul(t2[:], pq[:], sinT[:, csl])
                    nc.vector.tensor_add(dst, t1[:], t2[:])

                for step in range(len(blocks) + 1):
                    if step < len(blocks):
                        proj_stage1(blocks[step])
                    if step >= 1:
                        proj_stage2(blocks[step - 1])

            # ---- attention (head pair p, q-chunk n of 512) ----
            with tc.tile_pool(name="scp", bufs=2, space="PSUM") as scp, \
                 tc.tile_pool(name="cxp", bufs=2, space="PSUM") as cxp, \
                 tc.tile_pool(name="ep", bufs=4) as ep, \
                 tc.tile_pool(name="npl", bufs=2) as npl:
                for p in range(HT):
                    for n in range(2):
                        js = list(range(4 * n + 4))
                        qlo = 512 * n
                        cx0 = cxp.tile([65, 512], f32, tag="cx0", name="cx0")
                        cx1 = cxp.tile([65, 512], f32, tag="cx1", name="cx1")
                        es = {}

                        def emit_scores(j, n=n, p=p, qlo=qlo, es=es):
                            N = 512 if j < 4 * n else 512 - P * (j - 4 * n)
                            co = 512 - N
                            s_ps = scp.tile([P, 1024], f32, tag="s",
                                            name=f"s{p}_{n}_{j}")
                            for h in range(2):
                                rsl = slice(64 * h, 64 * (h + 1))
                                nc.tensor.matmul(
                                    s_ps[:, 512 * h:512 * h + N],
                                    kt[rsl, p, P * j:P * (j + 1)],
                                    qt[rsl, p, qlo + co:qlo + 512],
                                    start=True, stop=True,
                                    skip_group_check=True)
                            e = ep.tile([P, 1024], f16, tag="e",
                                        name=f"e{p}_{n}_{j}")
                            sv = s_ps.rearrange("q (a m) -> q a m", a=2)
                            ev = e.rearrange("q (a m) -> q a m", a=2)
                            nc.scalar.activation(ev[:, :, 0:N], sv[:, :, 0:N],
                                                 AF.Exp, scale=0.125)
                            if j >= 4 * n:
                                nc.vector.tensor_mul(e[:, 0:P], e[:, 0:P],
                                                     mj[:])
                                nc.vector.tensor_mul(e[:, 512:512 + P],
                                                     e[:, 512:512 + P], mj[:])
                            es[j] = e

                        def emit_ctx(j, n=n, p=p, js=js, es=es,
                                     cx0=cx0, cx1=cx1):
                            N = 512 if j < 4 * n else 512 - P * (j - 4 * n)
                            co = 512 - N
                            e = es.pop(j)
                            st, sp = (j == 0), (j == js[-1])
                            nc.tensor.matmul(cx0[0:65, co:512],
                                             v1r[:, j, p, 0:65],
                                             e[:, 0:N], start=st, stop=sp)
                            nc.tensor.matmul(cx1[0:65, co:512],
                                             v1r[:, j, p, 65:130],
                                             e[:, 512:512 + N],
                                             start=st, stop=sp)

                        for step in range(len(js) + 3):
                            if step < len(js):
                                emit_scores(js[step])
                            if step >= 3:
                                emit_ctx(js[step - 3])

                        # normalize: recip of both free denominator rows,
                        # one GpSimd broadcast to partitions 0:64, DVE muls;
                        # odd head's rows shift 0:64 -> 64:128 via SB->SB DMA
                        dn = npl.tile([1, 1024], f32, tag="dn", name="dn")
                        nc.vector.tensor_copy(dn[:, 0:512], cx0[64:65, :])
                        nc.vector.tensor_copy(dn[:, 512:1024], cx1[64:65, :])
                        r01 = npl.tile([1, 1024], f32, tag="r", name="r01")
                        nc.vector.reciprocal_approx_fast(r01[:], dn[:])
                        r01h = npl.tile([1, 1024], f16, tag="rh", name="r01h")
                        nc.vector.tensor_copy(r01h[:], r01[:])
                        # broadcast 1/den to 64 partitions via rank-1 matmul
                        rb = scp.tile([P, 1024], f32, tag="s", name="rb")
                        nc.tensor.matmul(rb[0:64, 0:512], ones[0:1, 0:64],
                                         r01h[:, 0:512], start=True,
                                         stop=True, skip_group_check=True)
                        nc.tensor.matmul(rb[0:64, 512:1024], ones[0:1, 0:64],
                                         r01h[:, 512:1024], start=True,
                                         stop=True, skip_group_check=True)
                        nrm = npl.tile([64, 1024], f32, tag="nrm", name="nrm")
                        nc.vector.tensor_copy(nrm[:], rb[0:64, :])
                        if taps:
                            g = 2 * p + n
                            dcp = npl.tile([1, 1024], f32, tag="dcp",
                                           name="dcp")
                            nc.vector.tensor_copy(dcp[:, 0:512],
                                                  cx0[64:65, :])
                            nc.vector.tensor_copy(dcp[:, 512:1024],
                                                  cx1[64:65, :])
                            nc.sync.dma_start(tap_ext["den"][g:g + 1, :],
                                              dcp[:])
                            nc.sync.dma_start(tap_ext["r01"][g:g + 1, :],
                                              r01[:])
                            nc.sync.dma_start(tap_ext["nrm"][:, g, :],
                                              nrm[:])
                        qsl = slice(qlo, qlo + 512)
                        nc.vector.tensor_mul(cn[0:64, p, qsl], cx0[0:64, :],
                                             nrm[:, 0:512])
                        stg = npl.tile([64, 512], f16, tag="stg", name="stg")
                        nc.vector.tensor_mul(stg[:], cx1[0:64, :],
                                             nrm[:, 512:1024])
                        nc.gpsimd.dma_start(cn[64:P, p, qsl], stg[:])

            if taps:
                for tn, tile_ap in (("qt", qt), ("kt", kt), ("v1", v1),
                                    ("cn", cn)):
                    nc.sync.dma_start(tap_ext[tn][:], tile_ap[:])

            # ---- output projection (partial: contract this core's 512) ----
            with tc.tile_pool(name="op", bufs=4, space="PSUM") as op, \
                 tc.tile_pool(name="ob", bufs=4) as ob:
                for i in range(8):
                    for m in range(2):
                        csl = slice(512 * m, 512 * (m + 1))
                        yp = op.tile([P, 512], f32, tag="yp", name="yp")
                        for t in range(HT):
                            nc.tensor.matmul(yp[:], cn[:, t, P * i:P * (i + 1)],
                                             wo[:, t, csl],
                                             start=(t == 0), stop=(t == HT - 1))
                        ys = ob.tile([P, 512], f16, tag="ys", name="ys")
                        nc.vector.tensor_copy(ys[:], yp[:])
                        nc.gpsimd.dma_start(y_sh[P * i:P * (i + 1), csl],
                                            ys[:])

    nc.compile()
    return nc


def _host_tables():
    # RoPE tables, computed in float32 to match the reference's jnp path.
    pos = np.arange(S, dtype=np.float32)
    inv = np.exp(np.arange(0, Dh, 2, dtype=np.float32)
                 * np.float32(-np.log(10000.0) / Dh))          # [32]
    ang = pos[:, None] * inv[None, :]                          # [S, 32]
    sin = np.sin(ang).astype(np.float32)
    cos = np.cos(ang).astype(np.float32)
    # per-partition pattern for [2 heads x 64, s] transposed layout
    dd = np.arange(P) % Dh
    cosP = np.empty((P, S), np.float32)
    sinP = np.empty((P, S), np.float32)
    lo = dd < 32
    cosP[lo] = cos[:, dd[lo]].T
    sinP[lo] = -sin[:, dd[lo]].T
    cosP[~lo] = cos[:, dd[~lo] - 32].T
    sinP[~lo] = sin[:, dd[~lo] - 32].T
    return cosP.astype(np.float16), sinP.astype(np.float16)


def _perm128():
    p = np.zeros((P, P), np.float16)
    i = np.arange(P)
    p[i, i ^ 32] = np.float16(1.0)
    return p


def _tile_T(a):
    # [rows, D] -> [P, KT, rows]: partition-tiled transpose for SBUF layout
    rows = a.shape[0]
    return np.ascontiguousarray(a.T.reshape(KT, P, rows).transpose(1, 0, 2))


def make_in_maps(x, Wq, bq, Wk, bk, Wv, bv, Wo, bo):
    x = np.asarray(x, np.float16)
    Wq = np.asarray(Wq, np.float16)
    Wk = np.asarray(Wk, np.float16)
    Wv = np.asarray(Wv, np.float16)
    Wo = np.asarray(Wo, np.float16)
    cosP, sinP = _host_tables()
    r = np.arange(P)[:, None]
    c = np.arange(P)[None, :]
    shared = {
        "cosT": cosP,
        "sinT": sinP,
        "mj": (r <= c).astype(np.float16),
        "p128": _perm128(),
    }

    in_maps = []
    for core in range(N_CORES):
        b, hh = core // 2, core % 2
        hsl = slice(512 * hh, 512 * hh + 512)
        m = {
            "xt_sh": _tile_T(x[b]),
            "wq": np.ascontiguousarray(Wq[:, hsl]),
            "wk": np.ascontiguousarray(Wk[:, hsl]),
            "wv": np.ascontiguousarray(Wv[:, hsl]),
            "wo": np.ascontiguousarray(Wo[hsl, :]),
            "bqt": np.ascontiguousarray(
                np.asarray(bq[hsl], np.float16).astype(np.float32)
                .reshape(HT, P).T),
            "bkt": np.ascontiguousarray(
                np.asarray(bk[hsl], np.float16).astype(np.float32)
                .reshape(HT, P).T),
            "bv": np.asarray(bv[hsl], np.float16).reshape(1, 512),
        }
        m.update(shared)
        in_maps.append(m)
    return in_maps


def kernel(x, Wq, bq, Wk, bk, Wv, bv, Wo, bo):
    from concourse.bass_utils import run_bass_kernel_spmd

    with _lock:
        if "nc" not in _cache:
            _cache["nc"] = _build_program()
    nc = _cache["nc"]

    in_maps = make_in_maps(x, Wq, bq, Wk, bk, Wv, bv, Wo, bo)
    res = run_bass_kernel_spmd(nc, in_maps, list(range(N_CORES)))

    bo32 = np.asarray(bo, np.float16).astype(np.float32)
    out = np.empty((B, S, D), np.float16)
    for b in range(B):
        y0 = res.results[2 * b]["y_sh"].astype(np.float32)
        y1 = res.results[2 * b + 1]["y_sh"].astype(np.float32)
        out[b] = (y0 + y1 + bo32).astype(np.float16)
    return out


# revision 35
# speedup vs baseline: 1.1148x; 1.1148x over previous
"""Trainium2 Bass kernel for CustomMultiHeadAttention (B=4, S=1024, D=1024, H=16, Dh=64).

Sharding: 8 cores = (batch b in 0..3) x (head-half hh in 0..1).
Core (b, hh) computes heads 8*hh..8*hh+7 of batch b over the FULL
sequence (natural q order), producing a partial output
y_part = ctx_half @ Wo[512*hh:512*hh+512, :]; the host sums the two
partials per batch (and adds bo).  This halves projection FLOPs and
weight DMA versus data-parallel-over-queries.

Pipeline (transposed layout, PE-centric):
  QT = rope(Wq^T x^T), KT = rope(Wk^T x^T)  - rope via permutation-matmul + DVE
  per head pair p (4) x q-chunk n (2): scT[kv,q] = KT_h^T QT_h,
  exp on ScalarE (scale=1/8), causal mask on the diagonal 128-col block,
  ctx accumulates with lhsT = [V_h0|1|V_h1] slots: h0 gets a free
  denominator row (M=65), h1's denominator is one M=1 matmul into the
  unused row 0 of its ctx PSUM bank.  Normalization reciprocals are
  partition-broadcast on GpSimd (no PE broadcast matmul), then
  cn = ctx * recip on DVE; out = cn^T Wo_half.
"""

import threading

import numpy as np

B, S, D, H, Dh = 4, 1024, 1024, 16, 64
P = 128
N_CORES = 8
KT = 8    # k (din) tiles
HT = 4    # dout tiles per core (8 heads = 512 dims)
ST = 8    # s tiles
VS2 = 130  # V pair slot: [V_h0(64) | 1 | V_h1(64) | 1]

_cache = {}
_lock = threading.Lock()


def _build_program(taps=False):
    import concourse.bass as bass  # noqa: F401
    import concourse.mybir as mybir
    import concourse.tile as tile
    from concourse import bacc

    dt = mybir.dt
    f16, f32 = dt.float16, dt.float32
    AF = mybir.ActivationFunctionType

    nc = bacc.Bacc("TRN2", target_bir_lowering=False, debug=False,
                   num_devices=N_CORES)

    def ein(name, shape):
        return nc.dram_tensor(name, shape, f16, kind="ExternalInput").ap()

    xt_sh = ein("xt_sh", [P, KT, S])      # x[b]^T, host-transposed
    wq_e = ein("wq", [D, 512])            # Wq[:, half-cols]
    wk_e = ein("wk", [D, 512])
    wv_e = ein("wv", [D, 512])
    wo_e = ein("wo", [512, D])            # Wo[half-rows, :]
    bqt_e = nc.dram_tensor("bqt", [P, HT], f32, kind="ExternalInput").ap()
    bkt_e = nc.dram_tensor("bkt", [P, HT], f32, kind="ExternalInput").ap()
    bv_e = ein("bv", [1, 512])
    cos_e = ein("cosT", [P, S])
    sin_e = ein("sinT", [P, S])
    mj_e = ein("mj", [P, P])
    p128_e = ein("p128", [P, P])
    y_sh = nc.dram_tensor("y_sh", [S, D], f16, kind="ExternalOutput").ap()
    tap_ext = {}
    if taps:
        for tn, shape in (("qt", [P, HT, S]), ("kt", [P, HT, S]),
                          ("v1", [P, ST, HT * VS2]), ("cn", [P, HT, S])):
            tap_ext[tn] = nc.dram_tensor("dbg_" + tn, shape, f16,
                                         kind="ExternalOutput").ap()
        tap_ext["nrm"] = nc.dram_tensor("dbg_nrm", [64, 8, 1024], f32,
                                        kind="ExternalOutput").ap()
        tap_ext["den"] = nc.dram_tensor("dbg_den", [8, 1024], f32,
                                        kind="ExternalOutput").ap()
        tap_ext["r01"] = nc.dram_tensor("dbg_r01", [8, 1024], f32,
                                        kind="ExternalOutput").ap()

    with tile.TileContext(nc) as tc:
        from contextlib import ExitStack
        with ExitStack() as ctx:
            big = ctx.enter_context(tc.tile_pool(name="big", bufs=1))

            xT = big.tile([P, KT, S], f16, tag="xT")       # x^T  [din, s]
            wq = big.tile([P, KT, 512], f16, tag="wq")
            wk = big.tile([P, KT, 512], f16, tag="wk")
            wv = big.tile([P, KT, 512], f16, tag="wv")
            wo = big.tile([P, HT, D], f16, tag="wo")
            bqt = big.tile([P, HT], f32, tag="bqt")
            bkt = big.tile([P, HT], f32, tag="bkt")
            bv_sb = big.tile([1, 512], f16, tag="bv")
            qt = big.tile([P, HT, S], f16, tag="qt")       # rope'd Q^T
            kt = big.tile([P, HT, S], f16, tag="kt")       # rope'd K^T
            v1 = big.tile([P, ST, HT * VS2], f16, tag="v1")
            cn = big.tile([P, HT, S], f16, tag="cn")       # normalized ctx^T
            cosT = big.tile([P, S], f16, tag="cosT")
            sinT = big.tile([P, S], f16, tag="sinT")
            mj = big.tile([P, P], f16, tag="mj")
            p128 = big.tile([P, P], f16, tag="p128")
            ones = big.tile([P, P], f16, tag="ones")

            # ---- input DMAs ----
            # sync queue: strict priority order - wq[k] and the c0 column
            # half of x[k] pairwise so the first Q rope chains start ~3us
            # in; everything small rides the gpsimd queue in parallel.
            for k in range(KT):
                nc.sync.dma_start(wq[:, k, :], wq_e[P * k:P * (k + 1), :])
                nc.sync.dma_start(xT[:, k, 0:512], xt_sh[:, k, 0:512])
            for k in range(KT):
                nc.sync.dma_start(xT[:, k, 512:1024], xt_sh[:, k, 512:1024])
            for k in range(KT):
                nc.sync.dma_start(wk[:, k, :], wk_e[P * k:P * (k + 1), :])
            for k in range(KT):
                nc.sync.dma_start(wv[:, k, :], wv_e[P * k:P * (k + 1), :])
            nc.sync.dma_start(bv_sb[:], bv_e[:])
            for t in range(HT):
                nc.sync.dma_start(wo[:, t, :], wo_e[P * t:P * (t + 1), :])
            for t, e in ((p128, p128_e), (cosT, cos_e), (sinT, sin_e),
                         (bqt, bqt_e), (bkt, bkt_e), (mj, mj_e)):
                nc.gpsimd.dma_start(t[:], e[:])
            nc.any.memset(ones[:], 1.0)
            v1r = v1.rearrange("p t (pr c) -> p t pr c", c=VS2)
            nc.any.memset(v1r[:, :, :, 64:65], 1.0)
            nc.any.memset(v1r[:, :, :, 129:130], 1.0)

            # ---- projections + rope, software-pipelined per column-half --
            with tc.tile_pool(name="pp", bufs=3, space="PSUM") as pp, \
                 tc.tile_pool(name="sc", bufs=4) as sc:
                # stage 1: k-chain matmuls into ps; stage 2 (lagged by one
                # block): perm matmul + rope DVE tail, so the PE never waits
                # on the DVE cast of the current block.
                blocks = []
                for c in range(2):
                    for t in range(HT):
                        blocks.append(("q", t, c))
                for c in range(2):
                    for t in range(HT):
                        blocks.append(("k", t, c))
                for t in range(ST):
                    blocks.append(("v", t, 0))

                state = {}

                def proj_stage1(blk):
                    kind, t, c = blk
                    csl = slice(512 * c, 512 * (c + 1))
                    dst_sl = slice(P * t, P * (t + 1))
                    ps = pp.tile([P, 512], f32, tag="ps", name="ps")
                    if kind == "v":
                        for k in range(KT):
                            nc.tensor.matmul(ps[:], xT[:, k, dst_sl],
                                             wv[:, k, :],
                                             start=(k == 0), stop=False)
                        nc.tensor.matmul(ps[:], ones[0:1, 0:P],
                                         bv_sb[0:1, :],
                                         start=False, stop=True)
                        state[blk] = ps
                        return
                    w_sb = wq if kind == "q" else wk
                    bias = bqt if kind == "q" else bkt
                    for k in range(KT):
                        nc.tensor.matmul(ps[:], w_sb[:, k, dst_sl],
                                         xT[:, k, csl],
                                         start=(k == 0), stop=(k == KT - 1))
                    raw = sc.tile([P, 512], f16, tag="raw", name="raw")
                    nc.vector.tensor_scalar_add(raw[:], ps[:],
                                                bias[:, t:t + 1])
                    state[blk] = raw

                def proj_stage2(blk):
                    kind, t, c = blk
                    csl = slice(512 * c, 512 * (c + 1))
                    if kind == "v":
                        vp = state.pop(blk)
                        vpr = vp.rearrange("p (pr two cc) -> p pr two cc",
                                           two=2, cc=64)
                        nc.vector.tensor_copy(v1r[:, t, :, 0:64],
                                              vpr[:, :, 0, :])
                        nc.vector.tensor_copy(v1r[:, t, :, 65:129],
                                              vpr[:, :, 1, :])
                        return
                    raw = state.pop(blk)
                    dst = (qt if kind == "q" else kt)[:, t, csl]
                    pq = pp.tile([P, 512], f32, tag="pq", name="pq")
                    nc.tensor.matmul(pq[:], p128[:], raw[:],
                                     start=True, stop=True)
                    t1 = sc.tile([P, 512], f16, tag="t1", name="t1")
                    nc.vector.tensor_mul(t1[:], raw[:], cosT[:, csl])
                    t2 = sc.tile([P, 512], f16, tag="t2", name="t2")
                    nc.vector.tensor_mul(t2[:], pq[:], sinT[:, csl])
                    nc.vector.tensor_add(dst, t1[:], t2[:])

                for step in range(len(blocks) + 1):
                    if step < len(blocks):
                        proj_stage1(blocks[step])
                    if step >= 1:
                        proj_stage2(blocks[step - 1])

            # ---- attention (head pair p, q-chunk n of 512) ----
            with tc.tile_pool(name="scp", bufs=2, space="PSUM") as scp, \
                 tc.tile_pool(name="cxp", bufs=2, space="PSUM") as cxp, \
                 tc.tile_pool(name="ep", bufs=4) as ep, \
                 tc.tile_pool(name="npl", bufs=2) as npl:
                deferred = []

                def emit_normalize_tail(fin):
                    # group `fin` finished its ctx accumulation and the
                    # reciprocal row r01h is ready; broadcast + normalize.
                    p, n, cx0, cx1, r01h = fin
                    rb = scp.tile([P, 1024], f32, tag="s", name="rb")
                    nc.tensor.matmul(rb[0:64, 0:512], ones[0:1, 0:64],
                                     r01h[:, 0:512], start=True,
                                     stop=True, skip_group_check=True)
                    nc.tensor.matmul(rb[0:64, 512:1024], ones[0:1, 0:64],
                                     r01h[:, 512:1024], start=True,
                                     stop=True, skip_group_check=True)
                    nrm = npl.tile([64, 1024], f32, tag="nrm", name="nrm")
                    nc.vector.tensor_copy(nrm[:], rb[0:64, :])
                    qsl = slice(512 * n, 512 * n + 512)
                    nc.vector.tensor_mul(cn[0:64, p, qsl], cx0[0:64, :],
                                         nrm[:, 0:512])
                    stg = npl.tile([64, 512], f16, tag="stg", name="stg")
                    nc.vector.tensor_mul(stg[:], cx1[0:64, :],
                                         nrm[:, 512:1024])
                    nc.gpsimd.dma_start(cn[64:P, p, qsl], stg[:])

                for p in range(HT):
                    for n in range(2):
                        js = list(range(4 * n + 4))
                        qlo = 512 * n
                        cx0 = cxp.tile([65, 512], f32, tag="cx0", name="cx0")
                        cx1 = cxp.tile([65, 512], f32, tag="cx1", name="cx1")
                        es = {}

                        def emit_scores(j, n=n, p=p, qlo=qlo, es=es):
                            N = 512 if j < 4 * n else 512 - P * (j - 4 * n)
                            co = 512 - N
                            s_ps = scp.tile([P, 1024], f32, tag="s",
                                            name=f"s{p}_{n}_{j}")
                            for h in range(2):
                                rsl = slice(64 * h, 64 * (h + 1))
                                nc.tensor.matmul(
                                    s_ps[:, 512 * h:512 * h + N],
                                    kt[rsl, p, P * j:P * (j + 1)],
                                    qt[rsl, p, qlo + co:qlo + 512],
                                    start=True, stop=True,
                                    skip_group_check=True)
                            e = ep.tile([P, 1024], f16, tag="e",
                                        name=f"e{p}_{n}_{j}")
                            sv = s_ps.rearrange("q (a m) -> q a m", a=2)
                            ev = e.rearrange("q (a m) -> q a m", a=2)
                            nc.scalar.activation(ev[:, :, 0:N], sv[:, :, 0:N],
                                                 AF.Exp, scale=0.125)
                            if j >= 4 * n:
                                nc.vector.tensor_mul(e[:, 0:P], e[:, 0:P],
                                                     mj[:])
                                nc.vector.tensor_mul(e[:, 512:512 + P],
                                                     e[:, 512:512 + P], mj[:])
                            es[j] = e

                        def emit_ctx(j, n=n, p=p, js=js, es=es,
                                     cx0=cx0, cx1=cx1):
                            N = 512 if j < 4 * n else 512 - P * (j - 4 * n)
                            co = 512 - N
                            e = es.pop(j)
                            st, sp = (j == 0), (j == js[-1])
                            nc.tensor.matmul(cx0[0:65, co:512],
                                             v1r[:, j, p, 0:65],
                                             e[:, 0:N], start=st, stop=sp)
                            nc.tensor.matmul(cx1[0:65, co:512],
                                             v1r[:, j, p, 65:130],
                                             e[:, 512:512 + N],
                                             start=st, stop=sp)

                        for step in range(len(js) + 3):
                            if step < len(js):
                                emit_scores(js[step])
                            if step == 2 and deferred:
                                emit_normalize_tail(deferred.pop(0))
                            if step >= 3:
                                emit_ctx(js[step - 3])

                        # normalize, stage 1 (Vector only): reciprocal of
                        # the two free denominator rows.  The broadcast and
                        # multiplies are deferred into the next group's j
                        # loop so no PE instruction here waits on Vector.
                        dn = npl.tile([1, 1024], f32, tag="dn", name="dn")
                        nc.vector.tensor_copy(dn[:, 0:512], cx0[64:65, :])
                        nc.vector.tensor_copy(dn[:, 512:1024], cx1[64:65, :])
                        r01 = npl.tile([1, 1024], f32, tag="r", name="r01")
                        nc.vector.reciprocal_approx_fast(r01[:], dn[:])
                        r01h = npl.tile([1, 1024], f16, tag="rh", name="r01h")
                        nc.vector.tensor_copy(r01h[:], r01[:])
                        deferred.append((p, n, cx0, cx1, r01h))

                while deferred:
                    emit_normalize_tail(deferred.pop(0))

            if taps:
                for tn, tile_ap in (("qt", qt), ("kt", kt), ("v1", v1),
                                    ("cn", cn)):
                    nc.sync.dma_start(tap_ext[tn][:], tile_ap[:])

            # ---- output projection (partial: contract this core's 512) ----
            with tc.tile_pool(name="op", bufs=4, space="PSUM") as op, \
                 tc.tile_pool(name="ob", bufs=4) as ob:
                for i in range(8):
                    for m in range(2):
                        csl = slice(512 * m, 512 * (m + 1))
                        yp = op.tile([P, 512], f32, tag="yp", name="yp")
                        for t in range(HT):
                            nc.tensor.matmul(yp[:], cn[:, t, P * i:P * (i + 1)],
                                             wo[:, t, csl],
                                             start=(t == 0), stop=(t == HT - 1))
                        ys = ob.tile([P, 512], f16, tag="ys", name="ys")
                        nc.vector.tensor_copy(ys[:], yp[:])
                        nc.sync.dma_start(y_sh[P * i:P * (i + 1), csl], ys[:])

    nc.compile()
    return nc


def _host_tables():
    # RoPE tables, computed in float32 to match the reference's jnp path.
    pos = np.arange(S, dtype=np.float32)
    inv = np.exp(np.arange(0, Dh, 2, dtype=np.float32)
                 * np.float32(-np.log(10000.0) / Dh))          # [32]
    ang = pos[:, None] * inv[None, :]                          # [S, 32]
    sin = np.sin(ang).astype(np.float32)
    cos = np.cos(ang).astype(np.float32)
    # per-partition pattern for [2 heads x 64, s] transposed layout
    dd = np.arange(P) % Dh
    cosP = np.empty((P, S), np.float32)
    sinP = np.empty((P, S), np.float32)
    lo = dd < 32
    cosP[lo] = cos[:, dd[lo]].T
    sinP[lo] = -sin[:, dd[lo]].T
    cosP[~lo] = cos[:, dd[~lo] - 32].T
    sinP[~lo] = sin[:, dd[~lo] - 32].T
    return cosP.astype(np.float16), sinP.astype(np.float16)


def _perm128():
    p = np.zeros((P, P), np.float16)
    i = np.arange(P)
    p[i, i ^ 32] = np.float16(1.0)
    return p


def _tile_T(a):
    # [rows, D] -> [P, KT, rows]: partition-tiled transpose for SBUF layout
    rows = a.shape[0]
    return np.ascontiguousarray(a.T.reshape(KT, P, rows).transpose(1, 0, 2))


def make_in_maps(x, Wq, bq, Wk, bk, Wv, bv, Wo, bo):
    x = np.asarray(x, np.float16)
    Wq = np.asarray(Wq, np.float16)
    Wk = np.asarray(Wk, np.float16)
    Wv = np.asarray(Wv, np.float16)
    Wo = np.asarray(Wo, np.float16)
    cosP, sinP = _host_tables()
    r = np.arange(P)[:, None]
    c = np.arange(P)[None, :]
    shared = {
        "cosT": cosP,
        "sinT": sinP,
        "mj": (r <= c).astype(np.float16),
        "p128": _perm128(),
    }

    in_maps = []
    for core in range(N_CORES):
        b, hh = core // 2, core % 2
        hsl = slice(512 * hh, 512 * hh + 512)
        m = {
            "xt_sh": _tile_T(x[b]),
            "wq": np.ascontiguousarray(Wq[:, hsl]),
            "wk": np.ascontiguousarray(Wk[:, hsl]),
            "wv": np.ascontiguousarray(Wv[:, hsl]),
            "wo": np.ascontiguousarray(Wo[hsl, :]),
            "bqt": np.ascontiguousarray(
                np.asarray(bq[hsl], np.float16).astype(np.float32)
                .reshape(HT, P).T),
            "bkt": np.ascontiguousarray(
                np.asarray(bk[hsl], np.float16).astype(np.float32)
                .reshape(HT, P).T),
            "bv": np.asarray(bv[hsl], np.float16).reshape(1, 512),
        }
        m.update(shared)
        in_maps.append(m)
    return in_maps


def kernel(x, Wq, bq, Wk, bk, Wv, bv, Wo, bo):
    from concourse.bass_utils import run_bass_kernel_spmd

    with _lock:
        if "nc" not in _cache:
            _cache["nc"] = _build_program()
    nc = _cache["nc"]

    in_maps = make_in_maps(x, Wq, bq, Wk, bk, Wv, bv, Wo, bo)
    res = run_bass_kernel_spmd(nc, in_maps, list(range(N_CORES)))

    bo32 = np.asarray(bo, np.float16).astype(np.float32)
    out = np.empty((B, S, D), np.float16)
    for b in range(B):
        y0 = res.results[2 * b]["y_sh"].astype(np.float32)
        y1 = res.results[2 * b + 1]["y_sh"].astype(np.float32)
        out[b] = (y0 + y1 + bo32).astype(np.float16)
    return out


# revision 51
# speedup vs baseline: 1.3427x; 1.2045x over previous
"""Trainium2 Bass kernel for CustomMultiHeadAttention (B=4, S=1024, D=1024, H=16, Dh=64).

Sharding: 8 cores = (batch b in 0..3) x (head-half hh in 0..1).
Core (b, hh) computes heads 8*hh..8*hh+7 of batch b over the FULL
sequence (natural q order), producing a partial output
y_part = ctx_half @ Wo[512*hh:512*hh+512, :]; the host sums the two
partials per batch (and adds bo).  This halves projection FLOPs and
weight DMA versus data-parallel-over-queries.

Pipeline (transposed layout, PE-centric):
  QT = rope(Wq^T x^T), KT = rope(Wk^T x^T)  - rope via permutation-matmul + DVE
  per head pair p (4) x q-chunk n (2): scT[kv,q] = KT_h^T QT_h,
  exp on ScalarE (scale=1/8), causal mask on the diagonal 128-col block,
  ctx accumulates with lhsT = [V_h0|1|V_h1] slots: h0 gets a free
  denominator row (M=65), h1's denominator is one M=1 matmul into the
  unused row 0 of its ctx PSUM bank.  Normalization reciprocals are
  partition-broadcast on GpSimd (no PE broadcast matmul), then
  cn = ctx * recip on DVE; out = cn^T Wo_half.
"""

import threading

import numpy as np

B, S, D, H, Dh = 4, 1024, 1024, 16, 64
P = 128
N_CORES = 8
KT = 8    # k (din) tiles
HT = 4    # dout tiles per core (8 heads = 512 dims)
ST = 8    # s tiles
VS2 = 128  # V head slot: [ones(64) | V(64)] -> ctx matmul M=128 yields
           # 64 replicated denominator rows (0:64) + 64 ctx rows (64:128)

_cache = {}
_lock = threading.Lock()


def _build_program(taps=False):
    import concourse.bass as bass  # noqa: F401
    import concourse.mybir as mybir
    import concourse.tile as tile
    from concourse import bacc

    dt = mybir.dt
    f16, f32 = dt.float16, dt.float32
    AF = mybir.ActivationFunctionType

    nc = bacc.Bacc("TRN2", target_bir_lowering=False, debug=False,
                   num_devices=N_CORES)

    def ein(name, shape):
        return nc.dram_tensor(name, shape, f16, kind="ExternalInput").ap()

    xt_sh = ein("xt_sh", [P, KT, S])      # x[b]^T, host-transposed
    wq_e = ein("wq", [D, 512])            # Wq[:, half-cols]
    wk_e = ein("wk", [D, 512])
    wv_e = ein("wv", [D, 512])
    wo_e = ein("wo", [512, D])            # Wo[half-rows, :]
    bqt_e = nc.dram_tensor("bqt", [P, HT], f32, kind="ExternalInput").ap()
    bkt_e = nc.dram_tensor("bkt", [P, HT], f32, kind="ExternalInput").ap()
    bv_e = ein("bv", [1, 512])
    cos_e = ein("cosT", [P, S])
    sin_e = ein("sinT", [P, S])
    mj_e = ein("mj", [P, P])
    p128_e = ein("p128", [P, P])
    y_sh = nc.dram_tensor("y_sh", [S, D], f16, kind="ExternalOutput").ap()
    tap_ext = {}
    if taps:
        for tn, shape in (("qt", [P, HT, S]), ("kt", [P, HT, S]),
                          ("v1", [P, ST, 8 * VS2]), ("cn", [P, HT, S])):
            tap_ext[tn] = nc.dram_tensor("dbg_" + tn, shape, f16,
                                         kind="ExternalOutput").ap()
        tap_ext["nrm"] = nc.dram_tensor("dbg_nrm", [64, 8, 1024], f32,
                                        kind="ExternalOutput").ap()
        tap_ext["den"] = nc.dram_tensor("dbg_den", [8, 1024], f32,
                                        kind="ExternalOutput").ap()
        tap_ext["r01"] = nc.dram_tensor("dbg_r01", [8, 1024], f32,
                                        kind="ExternalOutput").ap()

    with tile.TileContext(nc) as tc:
        from contextlib import ExitStack
        with ExitStack() as ctx:
            big = ctx.enter_context(tc.tile_pool(name="big", bufs=1))

            xT = big.tile([P, KT, S], f16, tag="xT")       # x^T  [din, s]
            wq = big.tile([P, KT, 512], f16, tag="wq")
            wk = big.tile([P, KT, 512], f16, tag="wk")
            wv = big.tile([P, KT, 512], f16, tag="wv")
            wo = big.tile([P, HT, D], f16, tag="wo")
            bqt = big.tile([P, HT], f32, tag="bqt")
            bkt = big.tile([P, HT], f32, tag="bkt")
            bv_sb = big.tile([1, 512], f16, tag="bv")
            qt = big.tile([P, HT, S], f16, tag="qt")       # rope'd Q^T
            kt = big.tile([P, HT, S], f16, tag="kt")       # rope'd K^T
            v1 = big.tile([P, ST, 8 * VS2], f16, tag="v1")
            cn = big.tile([P, HT, S], f16, tag="cn")       # normalized ctx^T
            cosT = big.tile([P, S], f16, tag="cosT")
            sinT = big.tile([P, S], f16, tag="sinT")
            mj = big.tile([P, P], f16, tag="mj")
            p128 = big.tile([P, P], f16, tag="p128")
            ones = big.tile([P, P], f16, tag="ones")

            # ---- input DMAs ----
            # sync queue: strict priority order - wq[k] and the c0 column
            # half of x[k] pairwise so the first Q rope chains start ~3us
            # in; everything small rides the gpsimd queue in parallel.
            for k in range(KT):
                nc.sync.dma_start(wq[:, k, :], wq_e[P * k:P * (k + 1), :])
                nc.sync.dma_start(xT[:, k, 0:512], xt_sh[:, k, 0:512])
            for k in range(KT):
                nc.sync.dma_start(xT[:, k, 512:1024], xt_sh[:, k, 512:1024])
            for k in range(KT):
                nc.sync.dma_start(wk[:, k, :], wk_e[P * k:P * (k + 1), :])
            for k in range(KT):
                nc.sync.dma_start(wv[:, k, :], wv_e[P * k:P * (k + 1), :])
            nc.sync.dma_start(bv_sb[:], bv_e[:])
            for t in range(HT):
                nc.sync.dma_start(wo[:, t, :], wo_e[P * t:P * (t + 1), :])
            for t, e in ((p128, p128_e), (cosT, cos_e), (sinT, sin_e),
                         (bqt, bqt_e), (bkt, bkt_e), (mj, mj_e)):
                nc.gpsimd.dma_start(t[:], e[:])
            nc.any.memset(ones[:], 1.0)
            v1h = v1.rearrange("p t (h c) -> p t h c", c=VS2)
            nc.any.memset(v1h[:, :, :, 0:64], 1.0)

            # ---- projections + rope, software-pipelined per column-half --
            with tc.tile_pool(name="pp", bufs=3, space="PSUM") as pp, \
                 tc.tile_pool(name="sc", bufs=4) as sc:
                # stage 1: k-chain matmuls into ps; stage 2 (lagged by one
                # block): perm matmul + rope DVE tail, so the PE never waits
                # on the DVE cast of the current block.
                blocks = []
                for c in range(2):
                    for t in range(HT):
                        blocks.append(("q", t, c))
                for c in range(2):
                    for t in range(HT):
                        blocks.append(("k", t, c))
                for t in range(ST):
                    blocks.append(("v", t, 0))

                state = {}

                def proj_stage1(blk):
                    kind, t, c = blk
                    csl = slice(512 * c, 512 * (c + 1))
                    dst_sl = slice(P * t, P * (t + 1))
                    ps = pp.tile([P, 512], f32, tag="ps", name="ps")
                    if kind == "v":
                        for k in range(KT):
                            nc.tensor.matmul(ps[:], xT[:, k, dst_sl],
                                             wv[:, k, :],
                                             start=(k == 0), stop=False)
                        nc.tensor.matmul(ps[:], ones[0:1, 0:P],
                                         bv_sb[0:1, :],
                                         start=False, stop=True)
                        state[blk] = ps
                        return
                    w_sb = wq if kind == "q" else wk
                    bias = bqt if kind == "q" else bkt
                    for k in range(KT):
                        nc.tensor.matmul(ps[:], w_sb[:, k, dst_sl],
                                         xT[:, k, csl],
                                         start=(k == 0), stop=(k == KT - 1))
                    raw = sc.tile([P, 512], f16, tag="raw", name="raw")
                    nc.vector.tensor_scalar_add(raw[:], ps[:],
                                                bias[:, t:t + 1])
                    state[blk] = raw

                def proj_stage2(blk):
                    kind, t, c = blk
                    csl = slice(512 * c, 512 * (c + 1))
                    if kind == "v":
                        vp = state.pop(blk)
                        vpr = vp.rearrange("p (h cc) -> p h cc", cc=64)
                        nc.vector.tensor_copy(v1h[:, t, :, 64:128],
                                              vpr[:, :, :])
                        return
                    raw = state.pop(blk)
                    dst = (qt if kind == "q" else kt)[:, t, csl]
                    pq = pp.tile([P, 512], f32, tag="pq", name="pq")
                    nc.tensor.matmul(pq[:], p128[:], raw[:],
                                     start=True, stop=True)
                    t1 = sc.tile([P, 512], f16, tag="t1", name="t1")
                    nc.vector.tensor_mul(t1[:], raw[:], cosT[:, csl])
                    t2 = sc.tile([P, 512], f16, tag="t2", name="t2")
                    nc.vector.tensor_mul(t2[:], pq[:], sinT[:, csl])
                    nc.vector.tensor_add(dst, t1[:], t2[:])

                for step in range(len(blocks) + 1):
                    if step < len(blocks):
                        proj_stage1(blocks[step])
                    if step >= 1:
                        proj_stage2(blocks[step - 1])

            # ---- attention (head pair p, q-chunk n of 512) ----
            with tc.tile_pool(name="scp", bufs=2, space="PSUM") as scp, \
                 tc.tile_pool(name="cxp", bufs=2, space="PSUM") as cxp, \
                 tc.tile_pool(name="ep", bufs=6) as ep, \
                 tc.tile_pool(name="npl", bufs=2) as npl:
                deferred = []

                def emit_normalize_tail(fin):
                    # group `fin` finished ctx accumulation and its recips
                    # (rows 0:64 of nrm tiles) are ready: shift them to
                    # partitions 64:128 by SB->SB DMA, multiply, and place
                    # the even head's rows into cn[0:64] by a second shift.
                    p, n, cx0, cx1, nr0, nr1 = fin
                    nc.gpsimd.dma_start(nr0[64:P, :], nr0[0:64, :])
                    nc.gpsimd.dma_start(nr1[64:P, :], nr1[0:64, :])
                    qsl = slice(512 * n, 512 * n + 512)
                    stg = npl.tile([P, 512], f16, tag="stg", name="stg")
                    nc.vector.tensor_mul(stg[64:P, :], cx0[64:P, :],
                                         nr0[64:P, :])
                    nc.vector.tensor_mul(cn[64:P, p, qsl], cx1[64:P, :],
                                         nr1[64:P, :])
                    nc.gpsimd.dma_start(cn[0:64, p, qsl], stg[64:P, :])

                for p in range(HT):
                    for n in range(2):
                        js = list(range(4 * n + 4))
                        qlo = 512 * n
                        cx0 = cxp.tile([P, 512], f32, tag="cx0", name="cx0")
                        cx1 = cxp.tile([P, 512], f32, tag="cx1", name="cx1")
                        es = {}

                        def emit_scores(j, n=n, p=p, qlo=qlo, es=es):
                            N = 512 if j < 4 * n else 512 - P * (j - 4 * n)
                            co = 512 - N
                            s_ps = scp.tile([P, 1024], f32, tag="s",
                                            name=f"s{p}_{n}_{j}")
                            for h in range(2):
                                rsl = slice(64 * h, 64 * (h + 1))
                                nc.tensor.matmul(
                                    s_ps[:, 512 * h:512 * h + N],
                                    kt[rsl, p, P * j:P * (j + 1)],
                                    qt[rsl, p, qlo + co:qlo + 512],
                                    start=True, stop=True,
                                    skip_group_check=True)
                            e = ep.tile([P, 1024], f16, tag="e",
                                        name=f"e{p}_{n}_{j}")
                            sv = s_ps.rearrange("q (a m) -> q a m", a=2)
                            ev = e.rearrange("q (a m) -> q a m", a=2)
                            nc.scalar.activation(ev[:, :, 0:N], sv[:, :, 0:N],
                                                 AF.Exp, scale=0.125)
                            if j >= 4 * n:
                                nc.vector.tensor_mul(e[:, 0:P], e[:, 0:P],
                                                     mj[:])
                                nc.vector.tensor_mul(e[:, 512:512 + P],
                                                     e[:, 512:512 + P], mj[:])
                            es[j] = e

                        def emit_ctx(j, n=n, p=p, js=js, es=es,
                                     cx0=cx0, cx1=cx1):
                            N = 512 if j < 4 * n else 512 - P * (j - 4 * n)
                            co = 512 - N
                            e = es.pop(j)
                            st, sp = (j == 0), (j == js[-1])
                            nc.tensor.matmul(cx0[:, co:512],
                                             v1h[:, j, 2 * p, :],
                                             e[:, 0:N], start=st, stop=sp)
                            nc.tensor.matmul(cx1[:, co:512],
                                             v1h[:, j, 2 * p + 1, :],
                                             e[:, 512:512 + N],
                                             start=st, stop=sp)

                        for step in range(len(js) + 3):
                            if step < len(js):
                                emit_scores(js[step])
                            if step == 2 and deferred:
                                emit_normalize_tail(deferred.pop(0))
                            if step >= 3:
                                emit_ctx(js[step - 3])

                        # normalize, stage 1 (Vector): 64-lane reciprocals
                        # of the replicated denominator rows 0:64; the
                        # shifts and multiplies are deferred into the next
                        # group's j loop.
                        nr0 = npl.tile([P, 512], f32, tag="nr0", name="nr0")
                        nc.vector.reciprocal_approx_fast(nr0[0:64, :],
                                                         cx0[0:64, :])
                        nr1 = npl.tile([P, 512], f32, tag="nr1", name="nr1")
                        nc.vector.reciprocal_approx_fast(nr1[0:64, :],
                                                         cx1[0:64, :])
                        deferred.append((p, n, cx0, cx1, nr0, nr1))

                while deferred:
                    emit_normalize_tail(deferred.pop(0))

            if taps:
                for tn, tile_ap in (("qt", qt), ("kt", kt), ("v1", v1),
                                    ("cn", cn)):
                    nc.sync.dma_start(tap_ext[tn][:], tile_ap[:])

            # ---- output projection (partial: contract this core's 512) ----
            with tc.tile_pool(name="op", bufs=4, space="PSUM") as op, \
                 tc.tile_pool(name="ob", bufs=4) as ob:
                for i in range(8):
                    for m in range(2):
                        csl = slice(512 * m, 512 * (m + 1))
                        yp = op.tile([P, 512], f32, tag="yp", name="yp")
                        for t in range(HT):
                            nc.tensor.matmul(yp[:], cn[:, t, P * i:P * (i + 1)],
                                             wo[:, t, csl],
                                             start=(t == 0), stop=(t == HT - 1))
                        ys = ob.tile([P, 512], f16, tag="ys", name="ys")
                        nc.vector.tensor_copy(ys[:], yp[:])
                        nc.sync.dma_start(y_sh[P * i:P * (i + 1), csl], ys[:])

    nc.compile()
    return nc


def _host_tables():
    # RoPE tables, computed in float32 to match the reference's jnp path.
    pos = np.arange(S, dtype=np.float32)
    inv = np.exp(np.arange(0, Dh, 2, dtype=np.float32)
                 * np.float32(-np.log(10000.0) / Dh))          # [32]
    ang = pos[:, None] * inv[None, :]                          # [S, 32]
    sin = np.sin(ang).astype(np.float32)
    cos = np.cos(ang).astype(np.float32)
    # per-partition pattern for [2 heads x 64, s] transposed layout
    dd = np.arange(P) % Dh
    cosP = np.empty((P, S), np.float32)
    sinP = np.empty((P, S), np.float32)
    lo = dd < 32
    cosP[lo] = cos[:, dd[lo]].T
    sinP[lo] = -sin[:, dd[lo]].T
    cosP[~lo] = cos[:, dd[~lo] - 32].T
    sinP[~lo] = sin[:, dd[~lo] - 32].T
    return cosP.astype(np.float16), sinP.astype(np.float16)


def _perm128():
    p = np.zeros((P, P), np.float16)
    i = np.arange(P)
    p[i, i ^ 32] = np.float16(1.0)
    return p


def _tile_T(a):
    # [rows, D] -> [P, KT, rows]: partition-tiled transpose for SBUF layout
    rows = a.shape[0]
    return np.ascontiguousarray(a.T.reshape(KT, P, rows).transpose(1, 0, 2))


def make_in_maps(x, Wq, bq, Wk, bk, Wv, bv, Wo, bo):
    x = np.asarray(x, np.float16)
    Wq = np.asarray(Wq, np.float16)
    Wk = np.asarray(Wk, np.float16)
    Wv = np.asarray(Wv, np.float16)
    Wo = np.asarray(Wo, np.float16)
    cosP, sinP = _host_tables()
    r = np.arange(P)[:, None]
    c = np.arange(P)[None, :]
    shared = {
        "cosT": cosP,
        "sinT": sinP,
        "mj": (r <= c).astype(np.float16),
        "p128": _perm128(),
    }

    in_maps = []
    for core in range(N_CORES):
        b, hh = core // 2, core % 2
        hsl = slice(512 * hh, 512 * hh + 512)
        m = {
            "xt_sh": _tile_T(x[b]),
            "wq": np.ascontiguousarray(Wq[:, hsl]),
            "wk": np.ascontiguousarray(Wk[:, hsl]),
            "wv": np.ascontiguousarray(Wv[:, hsl]),
            "wo": np.ascontiguousarray(Wo[hsl, :]),
            "bqt": np.ascontiguousarray(
                np.asarray(bq[hsl], np.float16).astype(np.float32)
                .reshape(HT, P).T),
            "bkt": np.ascontiguousarray(
                np.asarray(bk[hsl], np.float16).astype(np.float32)
                .reshape(HT, P).T),
            "bv": np.asarray(bv[hsl], np.float16).reshape(1, 512),
        }
        m.update(shared)
        in_maps.append(m)
    return in_maps


def kernel(x, Wq, bq, Wk, bk, Wv, bv, Wo, bo):
    from concourse.bass_utils import run_bass_kernel_spmd

    with _lock:
        if "nc" not in _cache:
            _cache["nc"] = _build_program()
    nc = _cache["nc"]

    in_maps = make_in_maps(x, Wq, bq, Wk, bk, Wv, bv, Wo, bo)
    res = run_bass_kernel_spmd(nc, in_maps, list(range(N_CORES)))

    bo32 = np.asarray(bo, np.float16).astype(np.float32)
    out = np.empty((B, S, D), np.float16)
    for b in range(B):
        y0 = res.results[2 * b]["y_sh"].astype(np.float32)
        y1 = res.results[2 * b + 1]["y_sh"].astype(np.float32)
        out[b] = (y0 + y1 + bo32).astype(np.float16)
    return out
